# revision 3
# baseline (speedup 1.0000x reference)
"""Trainium2 Bass kernel for nn_CNN_24472723653055 (AdderNet CNN) — v2.

Data-parallel over 8 NeuronCores: 2 images per core. BN batch stats and the
global LayerNorm stats are synchronized with small AllReduces.

adder2d(out[p,c] = -sum_k |x[p,k] - w[c,k]|) via |w-x| = 2*relu(w-x) - w + x:
  * weights in SBUF transposed as [ci_partition, (tap, c)] fp16 tiles
  * elementwise relu(w - x) instructions are MERGED across the taps that
    share one input column (multi-dim APs; DVE tensor_scalar runs in 4x
    mode, ScalarE activation amortizes its 222-cycle SBUF access)
  * routing per column-group: DVE->PE, Act->PE, or a CHAIN route where a
    tunable range of output positions accumulates tap tiles in SBUF fp16
    via wide TensorTensor adds (2x) so each chained position needs only ONE
    PE reduction matmul instead of taps-many (PE was the bottleneck)
  * TensorE reduces over the ci partitions with a one-hot 2.0 stationary
    column, accumulating psum[p_row, c]
  * rank-1 corrections fold into the psum evacuation:
    Y = psum + xsum[p] (per-partition scalar) + (-wsum) (broadcast tile)
"""

import sys

sys.path.insert(0, "/opt/trn_rl_repo")

import numpy as np

N_CORES = 8
N_LOC = 2            # images per core
N_TOT = 16

C1, H1, W1 = 128, 196, 3
HO1 = 96
P1 = N_LOC * HO1 * W1          # 576
C2, HO2 = 256, 46
P2 = N_LOC * HO2 * W1          # 276
C3, HO3, WO3 = 384, 21, 2
P3 = N_LOC * HO3 * WO3         # 84
TAPS1, TAPS2 = 6, 6
TAPS3 = 12                     # (kh=6) x (kw=2)
KB3 = 24                       # 2 ci-blocks x 12 taps

EPS_BN = 1e-5
EPS_LN = 1e-5
EPS_L2 = 1e-12

# routing knobs
K2 = 8         # L2 DVE-chain positions per (n, wo) line (ho < K2)
KP2 = 0        # L2 Pool-chain positions per line (K2 <= ho < K2+KP2)
ACT_F2 = (2, 5)  # fraction of L2 PE-route column groups on ScalarE
K3 = 4         # L3 DVE-chain positions per (n, wo) line (ho < K3)
KP3 = 0        # L3 Pool-chain positions per line (K3 <= ho < K3+KP3)
ACT_F3 = (2, 5)  # fraction of L3 PE-route column groups on ScalarE

_BUILD_CACHE = {}


def build_program(single=False):
    """single=True builds a 1-core variant with collectives replaced by
    DRAM copies — only for TimelineSim cost-model analysis."""
    import concourse.bass as bass
    import concourse.bacc as bacc
    import concourse.tile as tile
    import concourse.mybir as mybir
    from concourse import masks

    dt = mybir.dt
    f32 = dt.float32
    f16 = dt.float16
    Alu = mybir.AluOpType
    Act = mybir.ActivationFunctionType

    nc = bacc.Bacc("TRN2", target_bir_lowering=False, debug=False,
                   num_devices=1 if single else N_CORES)

    # ------------------------------------------------------------------ I/O
    x_in = nc.dram_tensor("x_in", [1, N_LOC * H1 * W1], f32, kind="ExternalInput").ap()
    nw1t = nc.dram_tensor("nw1t", [1, TAPS1 * C1], f32, kind="ExternalInput").ap()
    w2t = nc.dram_tensor("w2t", [128, TAPS2 * C2], f16, kind="ExternalInput").ap()
    w3t = nc.dram_tensor("w3t", [128, KB3 * C3], f16, kind="ExternalInput").ap()
    wfcp = nc.dram_tensor("wfcp", [128, 6 * 3 * 42], f32, kind="ExternalInput").ap()
    g1_d = nc.dram_tensor("g1_d", [C1], f32, kind="ExternalInput").ap()
    b1_d = nc.dram_tensor("b1_d", [C1], f32, kind="ExternalInput").ap()
    g2_d = nc.dram_tensor("g2_d", [C2], f32, kind="ExternalInput").ap()
    b2_d = nc.dram_tensor("b2_d", [C2], f32, kind="ExternalInput").ap()
    g3_d = nc.dram_tensor("g3_d", [C3], f32, kind="ExternalInput").ap()
    b3_d = nc.dram_tensor("b3_d", [C3], f32, kind="ExternalInput").ap()
    bfc_d = nc.dram_tensor("bfc_d", [6], f32, kind="ExternalInput").ap()
    out_d = nc.dram_tensor("out", [1, N_LOC * 6], f32, kind="ExternalOutput").ap()

    groups = [list(range(N_CORES))]

    with tile.TileContext(nc) as tc:
        with tc.tile_pool(name="weights", bufs=1) as wp, \
             tc.tile_pool(name="acts", bufs=1) as ap_pool, \
             tc.tile_pool(name="consts", bufs=1) as cp, \
             tc.tile_pool(name="smalls", bufs=1) as sp, \
             tc.tile_pool(name="dram", bufs=1, space="DRAM") as dram:

            # ---------------------------------------------------- constants
            ones2 = cp.tile([128, 255], f32)       # one-hot 2.0 col bank
            nc.vector.memset(ones2[:], 0.0)
            nc.vector.memset(ones2[:, 127:128], 2.0)
            ones2h = cp.tile([128, 255], f16)
            nc.vector.tensor_copy(ones2h[:], ones2[:])
            ones_kh = cp.tile([128, 1], f16)
            nc.vector.memset(ones_kh[:], 1.0)
            ones_row = cp.tile([1, 512], f32)       # K=1 all-ones rows
            nc.vector.memset(ones_row[:], 1.0)
            ones_row_h = cp.tile([1, 512], f16)
            nc.vector.memset(ones_row_h[:], 1.0)
            ones_k = cp.tile([128, 1], f32)         # stats reduction lhsT
            nc.vector.memset(ones_k[:], 1.0)
            ident = cp.tile([128, 128], f32)
            masks.make_identity(nc, ident[:])

            # ---------------------------------------------------- weight DMAs
            x_sb = wp.tile([1, N_LOC * H1 * W1], f32)
            nc.sync.dma_start(x_sb[:], x_in)
            nw1_sb = wp.tile([1, TAPS1 * C1], f32)
            nc.sync.dma_start(nw1_sb[:], nw1t)
            w2t_sb = wp.tile([128, TAPS2 * C2], f16)
            nc.sync.dma_start(w2t_sb[:], w2t)
            w3t_sb = wp.tile([128, KB3 * C3], f16)
            nc.sync.dma_start(w3t_sb[:], w3t)
            wfc_sb = wp.tile([128, 6 * 3 * 42], f32)
            nc.sync.dma_start(wfc_sb[:], wfcp)
            bfc_sb = sp.tile([1, 6], f32)
            nc.gpsimd.dma_start(bfc_sb[:], bfc_d.rearrange("(one j) -> one j", one=1))

            gb1 = sp.tile([128, 2], f32)
            nc.gpsimd.dma_start(gb1[:, 0:1], g1_d.rearrange("(p one) -> p one", one=1))
            nc.gpsimd.dma_start(gb1[:, 1:2], b1_d.rearrange("(p one) -> p one", one=1))
            gb2 = [sp.tile([128, 2], f32, name=f"gb2_{cb}") for cb in range(2)]
            gb3 = [sp.tile([128, 2], f32, name=f"gb3_{cb}") for cb in range(3)]
            for cb in range(2):
                nc.gpsimd.dma_start(gb2[cb][:, 0:1],
                                    g2_d[cb * 128:(cb + 1) * 128].rearrange("(p one) -> p one", one=1))
                nc.gpsimd.dma_start(gb2[cb][:, 1:2],
                                    b2_d[cb * 128:(cb + 1) * 128].rearrange("(p one) -> p one", one=1))
            for cb in range(3):
                nc.gpsimd.dma_start(gb3[cb][:, 0:1],
                                    g3_d[cb * 128:(cb + 1) * 128].rearrange("(p one) -> p one", one=1))
                nc.gpsimd.dma_start(gb3[cb][:, 1:2],
                                    b3_d[cb * 128:(cb + 1) * 128].rearrange("(p one) -> p one", one=1))

            # persistent activation tensors
            accr = ap_pool.tile([128, P1], f32)     # layer1 sum relu(x-w), [c1, p1]
            acc1 = ap_pool.tile([128, P1], f32)     # layer1 sum |x-w|
            act1 = ap_pool.tile([128, P1], f32)
            nact1 = ap_pool.tile([128, P1], f32)
            y2 = ap_pool.tile([128, 3 * C2], f32)   # layer2 raw, [p-rows, (pb, c)]
            act2 = [ap_pool.tile([128, P2], f32, name=f"act2_{cb}") for cb in range(2)]
            nact2 = [ap_pool.tile([128, P2], f32, name=f"nact2_{cb}") for cb in range(2)]
            y3 = ap_pool.tile([128, C3], f32)       # layer3 raw, [p3-rows, c3]
            act3 = ap_pool.tile([128, 3 * P3], f32)  # [ci, (cb, p3)]
            wb2 = ap_pool.tile([128, C2], f32)      # -wsum2 broadcast
            wb3 = ap_pool.tile([128, C3], f32)      # -wsum3 broadcast
            xs2col = sp.tile([128, 3], f32)         # xsum2 as columns per p-block
            xs3col = sp.tile([128, 1], f32)

            # ---------------------------------------------------- helpers
            def allreduce(sbuf_src_aps, widths, name):
                total = sum(a.shape[0] * w for a, w in zip(sbuf_src_aps, widths))
                cin = dram.tile([1, total], f32, name=f"cc_in_{name}")
                cout = dram.tile([1, total], f32, name=f"cc_out_{name}")
                off = 0
                for a, w in zip(sbuf_src_aps, widths):
                    n = a.shape[0] * w
                    nc.gpsimd.dma_start(
                        cin[0:1, off:off + n].rearrange("one (p w) -> (one p) w", w=w), a)
                    off += n
                if single:
                    nc.gpsimd.dma_start(cout[:], cin[:])
                else:
                    nc.gpsimd.collective_compute(
                        "AllReduce", Alu.add, replica_groups=groups,
                        ins=[cin.opt()], outs=[cout.opt()])
                return cout

            def bn_affine(st_sum, st_sq, gb, n_bn, name):
                t_pool = sp
                mean = t_pool.tile([128, 1], f32, name=f"{name}_mean")
                msq = t_pool.tile([128, 1], f32, name=f"{name}_msq")
                m2 = t_pool.tile([128, 1], f32, name=f"{name}_m2")
                tv = t_pool.tile([128, 1], f32, name=f"{name}_tv")
                s_ = t_pool.tile([128, 1], f32, name=f"{name}_s")
                r0 = t_pool.tile([128, 1], f32, name=f"{name}_r0")
                r0sq = t_pool.tile([128, 1], f32, name=f"{name}_r0sq")
                av = t_pool.tile([128, 1], f32, name=f"{name}_av")
                bv = t_pool.tile([128, 1], f32, name=f"{name}_bv")
                rr = t_pool.tile([128, 1], f32, name=f"{name}_rr")
                gr = t_pool.tile([128, 1], f32, name=f"{name}_gr")
                scale = t_pool.tile([128, 1], f32, name=f"{name}_scale")
                bias = t_pool.tile([128, 1], f32, name=f"{name}_bias")
                inv = 1.0 / n_bn
                nc.vector.tensor_scalar(out=mean[:], in0=st_sum, scalar1=inv,
                                        scalar2=None, op0=Alu.mult)
                nc.vector.tensor_scalar(out=msq[:], in0=st_sq, scalar1=inv,
                                        scalar2=None, op0=Alu.mult)
                nc.vector.tensor_tensor(out=m2[:], in0=mean[:], in1=mean[:], op=Alu.mult)
                nc.vector.scalar_tensor_tensor(out=tv[:], in0=msq[:], scalar=EPS_BN,
                                               in1=m2[:], op0=Alu.add, op1=Alu.subtract)
                nc.scalar.activation(out=s_[:], in_=tv[:], func=Act.Sqrt)
                nc.vector.reciprocal(out=r0[:], in_=s_[:])
                # one Newton step for rsqrt accuracy: r = r0*(1.5 - 0.5*tv*r0^2)
                nc.vector.tensor_tensor(out=r0sq[:], in0=r0[:], in1=r0[:], op=Alu.mult)
                nc.vector.tensor_tensor(out=av[:], in0=tv[:], in1=r0sq[:], op=Alu.mult)
                nc.vector.tensor_scalar(out=bv[:], in0=av[:], scalar1=-0.5,
                                        scalar2=1.5, op0=Alu.mult, op1=Alu.add)
                nc.vector.tensor_tensor(out=rr[:], in0=r0[:], in1=bv[:], op=Alu.mult)
                nc.vector.tensor_tensor(out=gr[:], in0=gb[:, 0:1], in1=rr[:], op=Alu.mult)
                nc.vector.tensor_scalar(out=scale[:], in0=gr[:], scalar1=-1.0,
                                        scalar2=None, op0=Alu.mult)
                nc.vector.scalar_tensor_tensor(out=bias[:], in0=gr[:], scalar=mean[:],
                                               in1=gb[:, 1:2], op0=Alu.mult, op1=Alu.add)
                return scale, bias

            # =================================================== layer 1
            xv = x_sb.rearrange("one (n h w) -> one n h w", n=N_LOC, h=H1, w=W1)
            # fp16 copies so the L1 broadcast matmuls run 1 cycle/row (fp32
            # matmul is 4 cycles/row)
            xh_sb = wp.tile([1, N_LOC * H1 * W1], f16)
            nc.vector.tensor_copy(xh_sb[:], x_sb[:])
            nw1_h = wp.tile([1, TAPS1 * C1], f16)
            nc.vector.tensor_copy(nw1_h[:], nw1_sb[:])
            xvh = xh_sb.rearrange("one (n h w) -> one n h w", n=N_LOC, h=H1, w=W1)
            with tc.tile_pool(name="ps1", bufs=2, space="PSUM") as ps1, \
                 tc.tile_pool(name="pre1", bufs=2, space="PSUM") as pre1:
                for half in range(N_LOC):
                    for tap in range(TAPS1):
                        pk = ps1.tile([128, HO1 * W1], f32, tag="pk", name="pk")
                        xrow = xvh[0:1, half, tap:tap + 2 * HO1 - 1:2, :]
                        nc.tensor.matmul(pk[:, :], lhsT=(ones_row_h[0:1, 0:128]),
                                         rhs=(xrow), start=True, stop=False)
                        nc.tensor.matmul(pk[:, :],
                                         lhsT=(nw1_h[0:1, tap * C1:(tap + 1) * C1]),
                                         rhs=(ones_row_h[0:1, 0:HO1 * W1]),
                                         start=False, stop=True)
                        dst = accr[:, half * HO1 * W1:(half + 1) * HO1 * W1]
                        if tap == 0:
                            nc.vector.tensor_scalar(out=dst, in0=pk[:, :], scalar1=0.0,
                                                    scalar2=None, op0=Alu.max)
                        else:
                            nc.vector.scalar_tensor_tensor(out=dst, in0=pk[:, :],
                                                           scalar=0.0, in1=dst,
                                                           op0=Alu.max, op1=Alu.add)
                # corrections: acc1 = 2*accr + ws1[c] - xs1[p]
                ws1 = sp.tile([1, C1], f32)
                nc.vector.tensor_scalar(out=ws1[:], in0=nw1_sb[0:1, 0:C1],
                                        scalar1=-1.0, scalar2=None, op0=Alu.mult)
                for tap in range(1, TAPS1):
                    nc.vector.scalar_tensor_tensor(
                        out=ws1[:], in0=nw1_sb[0:1, tap * C1:(tap + 1) * C1],
                        scalar=-1.0, in1=ws1[:], op0=Alu.mult, op1=Alu.add)
                pw1 = pre1.tile([128, 288], f32, tag="pre1", name="pw1")
                nc.tensor.matmul(pw1[:, 0:1], lhsT=ws1[0:1, :],
                                 rhs=ones_row[0:1, 0:1], start=True, stop=True)
                ws1col = sp.tile([128, 1], f32)
                nc.vector.tensor_copy(ws1col[:], pw1[:, 0:1])
                xs1 = sp.tile([1, P1], f32)
                xs1v = xs1.rearrange("one (n h w) -> one n h w", n=N_LOC, h=HO1, w=W1)
                nc.vector.tensor_scalar(out=xs1v[:], in0=xv[0:1, :, 0:2 * HO1 - 1:2, :],
                                        scalar1=0.0, scalar2=None, op0=Alu.add)
                for tap in range(1, TAPS1):
                    nc.vector.tensor_tensor(out=xs1v[:], in0=xs1v[:],
                                            in1=xv[0:1, :, tap:tap + 2 * HO1 - 1:2, :],
                                            op=Alu.add)
                nc.vector.tensor_scalar(out=acc1[:], in0=accr[:], scalar1=2.0,
                                        scalar2=ws1col[:], op0=Alu.mult, op1=Alu.add)
                for half in range(N_LOC):
                    pxb = pre1.tile([128, 288], f32, tag="pre1", name="pxb")
                    nc.tensor.matmul(pxb[:, :], lhsT=(ones_row[0:1, 0:128]),
                                     rhs=(xs1[0:1, half * 288:(half + 1) * 288]),
                                     start=True, stop=True)
                    sl = acc1[:, half * 288:(half + 1) * 288]
                    nc.vector.tensor_tensor(out=sl, in0=sl, in1=pxb[:, :],
                                            op=Alu.subtract)

            # BN1 stats (local): per-channel sum & sumsq over free dim
            s1_sum = sp.tile([128, 1], f32)
            s1_sq = sp.tile([128, 1], f32)
            scr1 = ap_pool.tile([128, P1], f32)
            nc.vector.tensor_scalar(out=scr1[:], in0=acc1[:], scalar1=0.0, scalar2=None,
                                    op0=Alu.add, op1=Alu.add, accum_out=s1_sum[:])
            nc.scalar.activation(out=scr1[:], in_=acc1[:], func=Act.Square,
                                 accum_out=s1_sq[:])
            cc1 = allreduce([s1_sum[:], s1_sq[:]], [1, 1], "bn1")
            st1 = sp.tile([128, 2], f32)
            nc.gpsimd.dma_start(st1[:, 0:1],
                                cc1[0:1, 0:128].rearrange("one (p w) -> (one p) w", w=1))
            nc.gpsimd.dma_start(st1[:, 1:2],
                                cc1[0:1, 128:256].rearrange("one (p w) -> (one p) w", w=1))
            sc1, bi1 = bn_affine(st1[:, 0:1], st1[:, 1:2], gb1, N_TOT * HO1 * W1, "bn1")
            nc.scalar.activation(out=act1[:], in_=acc1[:], func=Act.Relu,
                                 scale=sc1[:], bias=bi1[:])
            nc.vector.tensor_scalar(out=nact1[:], in0=act1[:], scalar1=-1.0,
                                    scalar2=None, op0=Alu.mult)

            # =================================================== layer 2
            lines2 = [(n, wo) for n in range(N_LOC) for wo in range(W1)]
            with tc.tile_pool(name="ps2", bufs=1, space="PSUM") as ps2, \
                 tc.tile_pool(name="ps2s", bufs=1, space="PSUM") as ps2s, \
                 tc.tile_pool(name="pre2", bufs=2, space="PSUM") as pre2, \
                 tc.tile_pool(name="d2p", bufs=14) as d2p, \
                 tc.tile_pool(name="mx2p", bufs=6) as mx2p, \
                 tc.tile_pool(name="mp2p", bufs=6) as mp2p, \
                 tc.tile_pool(name="acc2p", bufs=1) as acc2p, \
                 tc.tile_pool(name="sq2p", bufs=2) as sq2p:
                # ---- corrections prelude
                pw2 = pre2.tile([128, C2], f32, tag="pre2", name="pw2")
                for tap in range(TAPS2):
                    nc.tensor.matmul(pw2[0:1, :], lhsT=ones_kh[:, 0:1],
                                     rhs=(w2t_sb[:, tap * C2:(tap + 1) * C2]),
                                     start=(tap == 0), stop=(tap == TAPS2 - 1))
                negw2 = sp.tile([1, C2], f32)
                nc.vector.tensor_scalar(out=negw2[:], in0=pw2[0:1, :], scalar1=-1.0,
                                        scalar2=None, op0=Alu.mult)
                pb2 = pre2.tile([128, C2], f32, tag="pre2", name="pb2")
                nc.tensor.matmul(pb2[:, :], lhsT=(ones_row[0:1, 0:128]),
                                 rhs=(negw2[0:1, :]), start=True, stop=True)
                nc.vector.tensor_copy(wb2[:], pb2[:, :])
                cs1 = sp.tile([1, P1], f32)
                for half in range(N_LOC):
                    pcs = pre2.tile([128, 288], f32, tag="pre2", name="pcs")
                    nc.tensor.matmul(pcs[0:1, 0:288], lhsT=(ones_k[:, 0:1]),
                                     rhs=(act1[:, half * 288:(half + 1) * 288]),
                                     start=True, stop=True)
                    nc.vector.tensor_copy(cs1[0:1, half * 288:(half + 1) * 288],
                                          pcs[0:1, 0:288])
                xs2 = sp.tile([1, P2], f32)
                cs1v = cs1.rearrange("one (n h w) -> one n h w", n=N_LOC, h=HO1, w=W1)
                xs2v = xs2.rearrange("one (n h w) -> one n h w", n=N_LOC, h=HO2, w=W1)
                nc.vector.tensor_scalar(out=xs2v[:], in0=cs1v[0:1, :, 0:2 * HO2 - 1:2, :],
                                        scalar1=0.0, scalar2=None, op0=Alu.add)
                for tap in range(1, TAPS2):
                    nc.vector.tensor_tensor(out=xs2v[:], in0=xs2v[:],
                                            in1=cs1v[0:1, :, tap:tap + 2 * HO2 - 1:2, :],
                                            op=Alu.add)
                px2 = pre2.tile([128, C2], f32, tag="pre2", name="px2")
                for pb in range(3):
                    rows = 128 if pb < 2 else P2 - 256
                    nc.tensor.matmul(px2[0:rows, pb:pb + 1],
                                     lhsT=xs2[0:1, pb * 128:pb * 128 + rows],
                                     rhs=ones_row[0:1, 0:1], start=True, stop=True,
                                     skip_group_check=True)
                    nc.vector.tensor_copy(xs2col[0:rows, pb:pb + 1],
                                          px2[0:rows, pb:pb + 1])
                # L3 weight-sum prelude hoisted here: independent of act2, so
                # it overlaps L2 instead of sitting in the L2->L3 sync trough
                pw3 = pre2.tile([128, C3], f32, tag="pre2w3", name="pw3", bufs=1)
                for kb in range(KB3):
                    nc.tensor.matmul(pw3[0:1, :], lhsT=ones_kh[:, 0:1],
                                     rhs=(w3t_sb[:, kb * C3:(kb + 1) * C3]),
                                     start=(kb == 0), stop=(kb == KB3 - 1))
                negw3 = sp.tile([1, C3], f32)
                nc.vector.tensor_scalar(out=negw3[:], in0=pw3[0:1, :], scalar1=-1.0,
                                        scalar2=None, op0=Alu.mult)
                pb3w = pre2.tile([128, C3], f32, tag="pre2w3", name="pb3w", bufs=1)
                nc.tensor.matmul(pb3w[:, :], lhsT=(ones_row[0:1, 0:128]),
                                 rhs=(negw3[0:1, :]), start=True, stop=True)
                nc.vector.tensor_copy(wb3[:], pb3w[:, :])

                # ---- main loop (merged groups + DVE-chain + Pool-chain)
                pts2 = [ps2.tile([128, C2], f32, name=f"pt2_{pb}") for pb in range(3)]
                acc2t = [acc2p.tile([128, K2 * C2], f16, name=f"acc2_{li}")
                         for li in range(len(lines2))]
                accp2t = [acc2p.tile([128, KP2 * C2], f16, name=f"accp2_{li}")
                          for li in range(len(lines2))] if KP2 else None
                w2v = w2t_sb.rearrange("p (t c) -> p t c", c=C2)

                def l2_taps(h):
                    return [t for t in range(TAPS2)
                            if (h - t) >= 0 and (h - t) % 2 == 0 and (h - t) // 2 < HO2]

                # pre-pass: matmul counts per psum block
                mm_tot2 = [0, 0, 0]
                for (n, wo) in lines2:
                    for h in range(HO1):
                        for t in l2_taps(h):
                            ho = (h - t) // 2
                            if ho >= K2 + KP2:
                                p = n * (HO2 * W1) + ho * W1 + wo
                                mm_tot2[p // 128] += 1
                    for ho in range(K2 + KP2):
                        p = n * (HO2 * W1) + ho * W1 + wo
                        mm_tot2[p // 128] += 1
                mm_done2 = [0, 0, 0]

                def emit_mm2(p, rhs):
                    pb, r = divmod(p, 128)
                    mm_done2[pb] += 1
                    nc.tensor.matmul(pts2[pb][:, :], lhsT=ones2h[:, 127 - r:255 - r],
                                     rhs=rhs, start=(mm_done2[pb] == 1),
                                     stop=(mm_done2[pb] == mm_tot2[pb]),
                                     skip_group_check=True)

                # interleave chain-band h (DVE/Pool-only work) with PE-band h
                # so no engine sits idle in a phase; each band stays ascending
                # so first-touch direct writes precede chain adds
                band_a = list(range(0, 2 * (K2 + KP2) + 4))
                band_b = list(range(2 * (K2 + KP2) + 4, HO1))
                h_order = []
                ia = ib = 0
                for i in range(HO1):
                    fa = ia / len(band_a)
                    fb = ib / len(band_b)
                    if ia < len(band_a) and (ib >= len(band_b) or fa <= fb):
                        h_order.append(band_a[ia]); ia += 1
                    else:
                        h_order.append(band_b[ib]); ib += 1

                gidx2 = 0
                for h in h_order:
                    for li, (n, wo) in enumerate(lines2):
                        col = n * (HO1 * W1) + h * W1 + wo
                        taps = l2_taps(h)
                        if not taps:
                            continue
                        chain_t = [t for t in taps if (h - t) // 2 < K2]
                        pool_t = [t for t in taps
                                  if K2 <= (h - t) // 2 < K2 + KP2]
                        pe_t = [t for t in taps if (h - t) // 2 >= K2 + KP2]
                        if pe_t:
                            G = len(pe_t)
                            dt_ = d2p.tile([128, G * C2], f16, tag="d2", name="d2")
                            dt3 = dt_.rearrange("p (t c) -> p t c", c=C2)
                            wv = w2v[:, pe_t[0]:pe_t[-1] + 1:2, :]
                            if (gidx2 * ACT_F2[0]) % ACT_F2[1] < ACT_F2[0]:
                                nc.scalar.activation(out=dt3[:], in_=wv, func=Act.Relu,
                                                     bias=nact1[:, col:col + 1])
                            else:
                                nc.vector.tensor_scalar(out=dt3[:], in0=wv,
                                                        scalar1=act1[:, col:col + 1],
                                                        scalar2=0.0, op0=Alu.subtract,
                                                        op1=Alu.max)
                            gidx2 += 1
                            for i, t in enumerate(pe_t):
                                p = n * (HO2 * W1) + ((h - t) // 2) * W1 + wo
                                emit_mm2(p, dt3[:, i, :])
                        if pool_t:
                            # Act+Pool chain: ScalarE produces relu(w-x), the
                            # GPSIMD Pool engine accumulates (TensorTensor add
                            # is the only elementwise op Pool supports)
                            pacc = accp2t[li]
                            direct = (pool_t[0] == 0)
                            st_t = pool_t[1:] if direct else pool_t
                            if direct:
                                ho = h // 2
                                off = (ho - K2) * C2
                                nc.vector.tensor_scalar(
                                    out=pacc[:, off:off + C2], in0=w2v[:, 0, :],
                                    scalar1=act1[:, col:col + 1], scalar2=0.0,
                                    op0=Alu.subtract, op1=Alu.max)
                            if st_t:
                                G = len(st_t)
                                mx = mp2p.tile([128, G * C2], f16, tag="mp2",
                                               name="mp2")
                                mx3 = mx.rearrange("p (t c) -> p t c", c=C2)
                                wv = w2v[:, st_t[0]:st_t[-1] + 1:2, :]
                                nc.vector.tensor_scalar(out=mx3[:], in0=wv,
                                                        scalar1=act1[:, col:col + 1],
                                                        scalar2=0.0,
                                                        op0=Alu.subtract,
                                                        op1=Alu.max)
                                ho_lo = (h - st_t[-1]) // 2
                                ho_hi = (h - st_t[0]) // 2
                                seg = pacc[:, (ho_lo - K2) * C2:
                                           (ho_hi - K2 + 1) * C2]
                                in1 = mx3[:, ::-1, :] if G > 1 else mx3[:, 0, :]
                                nc.gpsimd.tensor_tensor(out=seg, in0=seg,
                                                        in1=in1, op=Alu.add)
                        if chain_t:
                            lacc = acc2t[li]
                            direct = (chain_t[0] == 0)
                            mx_t = chain_t[1:] if direct else chain_t
                            if direct:
                                ho = h // 2
                                nc.vector.tensor_scalar(
                                    out=lacc[:, ho * C2:(ho + 1) * C2],
                                    in0=w2v[:, 0, :], scalar1=act1[:, col:col + 1],
                                    scalar2=0.0, op0=Alu.subtract, op1=Alu.max)
                            if mx_t:
                                G = len(mx_t)
                                mx = mx2p.tile([128, G * C2], f16, tag="mx2", name="mx2")
                                mx3 = mx.rearrange("p (t c) -> p t c", c=C2)
                                wv = w2v[:, mx_t[0]:mx_t[-1] + 1:2, :]
                                nc.vector.tensor_scalar(out=mx3[:], in0=wv,
                                                        scalar1=act1[:, col:col + 1],
                                                        scalar2=0.0, op0=Alu.subtract,
                                                        op1=Alu.max)
                                ho_lo = (h - mx_t[-1]) // 2
                                ho_hi = (h - mx_t[0]) // 2
                                seg = lacc[:, ho_lo * C2:(ho_hi + 1) * C2]
                                in1 = mx3[:, ::-1, :] if G > 1 else mx3[:, 0, :]
                                nc.vector.tensor_tensor(out=seg, in0=seg, in1=in1,
                                                        op=Alu.add)
                # deferred chain-completion matmuls: all chain adds are done by
                # now (in-order engines), so PE never waits on them here
                for li, (n, wo) in enumerate(lines2):
                    for ho in range(K2):
                        p = n * (HO2 * W1) + ho * W1 + wo
                        emit_mm2(p, acc2t[li][:, ho * C2:(ho + 1) * C2])
                    for ho in range(K2, K2 + KP2):
                        p = n * (HO2 * W1) + ho * W1 + wo
                        off = (ho - K2) * C2
                        emit_mm2(p, accp2t[li][:, off:off + C2])

                # evacuate with corrections + stats
                st2_sum = ps2s.tile([1, C2], f32)
                st2_sq = ps2s.tile([1, C2], f32)
                for pb in range(3):
                    rows = 128 if pb < 2 else P2 - 256
                    ysl = y2[0:rows, pb * C2:(pb + 1) * C2]
                    nc.vector.scalar_tensor_tensor(
                        out=ysl, in0=pts2[pb][0:rows, :],
                        scalar=xs2col[0:rows, pb:pb + 1],
                        in1=wb2[0:rows, :], op0=Alu.add, op1=Alu.add)
                    sq_t = sq2p.tile([128, C2], f32, tag="sq2", name="sq2")
                    nc.scalar.activation(out=sq_t[0:rows, :], in_=ysl, func=Act.Square)
                    nc.tensor.matmul(st2_sum[0:1, :], lhsT=(ones_k[0:rows, 0:1]),
                                     rhs=(ysl), start=(pb == 0), stop=(pb == 2),
                                     skip_group_check=True)
                    nc.tensor.matmul(st2_sq[0:1, :], lhsT=(ones_k[0:rows, 0:1]),
                                     rhs=(sq_t[0:rows, :]), start=(pb == 0), stop=(pb == 2),
                                     skip_group_check=True)
                st2_sb = sp.tile([1, 2 * C2], f32)
                nc.vector.tensor_copy(st2_sb[0:1, 0:C2], st2_sum[0:1, :])
                nc.vector.tensor_copy(st2_sb[0:1, C2:2 * C2], st2_sq[0:1, :])
                cc2 = allreduce([st2_sb[0:1, :]], [2 * C2], "bn2")
            st2 = sp.tile([128, 4], f32)
            for cb in range(2):
                nc.gpsimd.dma_start(
                    st2[:, cb:cb + 1],
                    cc2[0:1, cb * 128:(cb + 1) * 128].rearrange("one (p w) -> (one p) w", w=1))
                nc.gpsimd.dma_start(
                    st2[:, 2 + cb:3 + cb],
                    cc2[0:1, C2 + cb * 128:C2 + (cb + 1) * 128].rearrange("one (p w) -> (one p) w", w=1))
            with tc.tile_pool(name="pst2", bufs=2, space="PSUM") as pst2:
                # transposes depend only on y2 — run them while the BN2
                # allreduce is still in flight
                ptrs = []
                for cb in range(2):
                    ptr = pst2.tile([128, P2], f32, tag="pst2", name=f"pst2_{cb}")
                    for pb in range(3):
                        rows = 128 if pb < 2 else P2 - 256
                        nc.tensor.transpose(
                            ptr[:, pb * 128:pb * 128 + rows],
                            y2[0:rows, pb * C2 + cb * 128:pb * C2 + (cb + 1) * 128],
                            ident[0:rows, 0:rows])
                    ptrs.append(ptr)
                for cb in range(2):
                    sc2, bi2 = bn_affine(st2[:, cb:cb + 1], st2[:, 2 + cb:3 + cb],
                                         gb2[cb], N_TOT * HO2 * W1, f"bn2_{cb}")
                    nc.scalar.activation(out=act2[cb][:], in_=ptrs[cb][:, :],
                                         func=Act.Relu, scale=sc2[:], bias=bi2[:])
                    nc.vector.tensor_scalar(out=nact2[cb][:], in0=act2[cb][:],
                                            scalar1=-1.0, scalar2=None, op0=Alu.mult)

            # =================================================== layer 3
            with tc.tile_pool(name="ps3", bufs=1, space="PSUM") as ps3, \
                 tc.tile_pool(name="ps3s", bufs=1, space="PSUM") as ps3s, \
                 tc.tile_pool(name="pre3", bufs=2, space="PSUM") as pre3, \
                 tc.tile_pool(name="d3p", bufs=11) as d3p, \
                 tc.tile_pool(name="mx3p", bufs=6) as mx3p, \
                 tc.tile_pool(name="mp3p", bufs=4) as mp3p, \
                 tc.tile_pool(name="acc3p", bufs=1) as acc3p, \
                 tc.tile_pool(name="sq3p", bufs=1) as sq3p:
                # ---- corrections prelude (wb3 already computed during L2)
                cs2 = sp.tile([1, 2 * P2], f32)
                for cb in range(2):
                    pcs2 = pre3.tile([128, C3], f32, tag="pre3", name="pcs2")
                    nc.tensor.matmul(pcs2[0:1, 0:P2], lhsT=(ones_k[:, 0:1]),
                                     rhs=(act2[cb][:]), start=True, stop=True)
                    nc.vector.tensor_copy(cs2[0:1, cb * P2:(cb + 1) * P2],
                                          pcs2[0:1, 0:P2])
                xs3 = sp.tile([1, P3], f32)
                xs3v = xs3.rearrange("one (n h w) -> one n h w", n=N_LOC, h=HO3, w=WO3)
                first_x = True
                for cb in range(2):
                    csv = cs2[0:1, cb * P2:(cb + 1) * P2].rearrange(
                        "one (n h w) -> one n h w", n=N_LOC, h=HO2, w=W1)
                    for tap in range(TAPS3):
                        ki, kj = divmod(tap, 2)
                        view = csv[0:1, :, ki:ki + 2 * HO3 - 1:2, kj:kj + WO3]
                        if first_x:
                            nc.vector.tensor_scalar(out=xs3v[:], in0=view, scalar1=0.0,
                                                    scalar2=None, op0=Alu.add)
                            first_x = False
                        else:
                            nc.vector.tensor_tensor(out=xs3v[:], in0=xs3v[:],
                                                    in1=view, op=Alu.add)
                px3 = pre3.tile([128, C3], f32, tag="pre3", name="px3")
                nc.tensor.matmul(px3[0:P3, 0:1], lhsT=xs3[0:1, 0:P3],
                                 rhs=ones_row[0:1, 0:1], start=True, stop=True)
                nc.vector.tensor_copy(xs3col[0:P3, 0:1], px3[0:P3, 0:1])

                # ---- main loop (merged groups + DVE-chain + Pool-chain)
                pt3 = ps3.tile([128, C3], f32)
                acc3t = [[acc3p.tile([128, K3 * WO3 * C3], f16, name=f"acc3_{cib}_{n}")
                          for n in range(N_LOC)] for cib in range(2)]
                accp3t = [[acc3p.tile([128, KP3 * WO3 * C3], f16,
                                      name=f"accp3_{cib}_{n}")
                           for n in range(N_LOC)] for cib in range(2)] \
                    if KP3 else None
                w3v = w3t_sb.rearrange("p (cb ki kj c) -> p cb ki kj c",
                                       cb=2, ki=6, kj=2)

                def l3_ki(h2):
                    return [ki for ki in range(6)
                            if (h2 - ki) >= 0 and (h2 - ki) % 2 == 0
                            and (h2 - ki) // 2 < HO3]

                def l3_kj(w2):
                    return [kj for kj in range(2) if 0 <= w2 - kj < WO3]

                # pre-pass: total matmuls into pt3
                mm_tot3 = 0
                for h2 in range(HO2):
                    for w2 in range(W1):
                        nkj = len(l3_kj(w2))
                        for ki in l3_ki(h2):
                            if (h2 - ki) // 2 >= K3 + KP3:
                                mm_tot3 += nkj * 2 * N_LOC
                mm_tot3 += (K3 + KP3) * WO3 * N_LOC * 2   # one per (pos, cib)
                mm_done3 = [0]

                def emit_mm3(p, rhs):
                    mm_done3[0] += 1
                    nc.tensor.matmul(pt3[:, :], lhsT=ones2h[:, 127 - p:255 - p],
                                     rhs=rhs, start=(mm_done3[0] == 1),
                                     stop=(mm_done3[0] == mm_tot3),
                                     skip_group_check=True)

                band3_a = list(range(0, 2 * (K3 + KP3) + 4))
                band3_b = list(range(2 * (K3 + KP3) + 4, HO2))
                h2_order = []
                ia = ib = 0
                for i in range(HO2):
                    fa = ia / len(band3_a)
                    fb = ib / len(band3_b)
                    if ia < len(band3_a) and (ib >= len(band3_b) or fa <= fb):
                        h2_order.append(band3_a[ia]); ia += 1
                    else:
                        h2_order.append(band3_b[ib]); ib += 1

                gidx3 = 0
                for h2 in h2_order:
                    for w2 in range(W1):
                        kj_list = l3_kj(w2)
                        ki_list = l3_ki(h2)
                        if not ki_list or not kj_list:
                            continue
                        chain_ki = [ki for ki in ki_list if (h2 - ki) // 2 < K3]
                        pool_ki = [ki for ki in ki_list
                                   if K3 <= (h2 - ki) // 2 < K3 + KP3]
                        pe_ki = [ki for ki in ki_list
                                 if (h2 - ki) // 2 >= K3 + KP3]
                        for n in range(N_LOC):
                            col = n * (HO2 * W1) + h2 * W1 + w2
                            for cib in range(2):
                                if pe_ki:
                                    nki, nkj = len(pe_ki), len(kj_list)
                                    dt_ = d3p.tile([128, nki * nkj * C3], f16,
                                                   tag="d3", name="d3")
                                    dt4 = dt_.rearrange("p (i j c) -> p i j c",
                                                        j=nkj, c=C3)
                                    wv = w3v[:, cib, pe_ki[0]:pe_ki[-1] + 1:2,
                                             kj_list[0]:kj_list[-1] + 1, :]
                                    if (gidx3 * ACT_F3[0]) % ACT_F3[1] < ACT_F3[0]:
                                        nc.scalar.activation(
                                            out=dt4[:], in_=wv, func=Act.Relu,
                                            bias=nact2[cib][:, col:col + 1])
                                    else:
                                        nc.vector.tensor_scalar(
                                            out=dt4[:], in0=wv,
                                            scalar1=act2[cib][:, col:col + 1],
                                            scalar2=0.0, op0=Alu.subtract, op1=Alu.max)
                                    gidx3 += 1
                                    for i, ki in enumerate(pe_ki):
                                        for j, kj in enumerate(kj_list):
                                            p = n * (HO3 * WO3) \
                                                + ((h2 - ki) // 2) * WO3 + (w2 - kj)
                                            emit_mm3(p, dt4[:, i, j, :])
                                if pool_ki:
                                    pacc = accp3t[cib][n]
                                    paccv = pacc.rearrange(
                                        "p (ho wo c) -> p ho wo c", wo=WO3, c=C3)
                                    prects = []
                                    if pool_ki[0] == 0 and kj_list[0] == 0:
                                        ho, wo = h2 // 2, w2
                                        nc.scalar.activation(
                                            out=paccv[:, ho - K3, wo, :],
                                            in_=w3v[:, cib, 0, 0, :],
                                            func=Act.Relu,
                                            bias=nact2[cib][:, col:col + 1])
                                        if len(kj_list) > 1:
                                            prects.append(([0], kj_list[1:]))
                                        if len(pool_ki) > 1:
                                            prects.append((pool_ki[1:], kj_list))
                                    else:
                                        prects.append((pool_ki, kj_list))
                                    for (kis, kjs) in prects:
                                        nki, nkj = len(kis), len(kjs)
                                        mx = mp3p.tile([128, nki * nkj * C3], f16,
                                                       tag="mp3", name="mp3")
                                        mx4 = mx.rearrange("p (i j c) -> p i j c",
                                                           j=nkj, c=C3)
                                        wv = w3v[:, cib, kis[0]:kis[-1] + 1:2,
                                                 kjs[0]:kjs[-1] + 1, :]
                                        nc.scalar.activation(
                                            out=mx4[:], in_=wv, func=Act.Relu,
                                            bias=nact2[cib][:, col:col + 1])
                                        ho_lo = (h2 - kis[-1]) // 2
                                        ho_hi = (h2 - kis[0]) // 2
                                        # Pool TensorTensor: keep <=3D, one op
                                        # per kj
                                        for j, kj in enumerate(kjs):
                                            wo = w2 - kj
                                            seg = paccv[:, ho_lo - K3:
                                                        ho_hi - K3 + 1, wo, :]
                                            in1 = mx4[:, ::-1, j, :] \
                                                if nki > 1 else mx4[:, 0, j, :]
                                            nc.gpsimd.tensor_tensor(
                                                out=seg, in0=seg, in1=in1,
                                                op=Alu.add)
                                if chain_ki:
                                    lacc = acc3t[cib][n]
                                    laccv = lacc.rearrange("p (ho wo c) -> p ho wo c",
                                                           wo=WO3, c=C3)
                                    rects = []
                                    if chain_ki[0] == 0 and kj_list[0] == 0:
                                        # direct first-touch (ki=0, kj=0)
                                        ho, wo = h2 // 2, w2
                                        nc.vector.tensor_scalar(
                                            out=laccv[:, ho, wo, :],
                                            in0=w3v[:, cib, 0, 0, :],
                                            scalar1=act2[cib][:, col:col + 1],
                                            scalar2=0.0, op0=Alu.subtract, op1=Alu.max)
                                        if len(kj_list) > 1:
                                            rects.append(([0], kj_list[1:]))
                                        if len(chain_ki) > 1:
                                            rects.append((chain_ki[1:], kj_list))
                                    else:
                                        rects.append((chain_ki, kj_list))
                                    for (kis, kjs) in rects:
                                        nki, nkj = len(kis), len(kjs)
                                        mx = mx3p.tile([128, nki * nkj * C3], f16,
                                                       tag="mx3", name="mx3")
                                        mx4 = mx.rearrange("p (i j c) -> p i j c",
                                                           j=nkj, c=C3)
                                        wv = w3v[:, cib, kis[0]:kis[-1] + 1:2,
                                                 kjs[0]:kjs[-1] + 1, :]
                                        nc.vector.tensor_scalar(
                                            out=mx4[:], in0=wv,
                                            scalar1=act2[cib][:, col:col + 1],
                                            scalar2=0.0, op0=Alu.subtract, op1=Alu.max)
                                        ho_lo = (h2 - kis[-1]) // 2
                                        ho_hi = (h2 - kis[0]) // 2
                                        wo_lo = w2 - kjs[-1]
                                        wo_hi = w2 - kjs[0]
                                        seg = laccv[:, ho_lo:ho_hi + 1,
                                                    wo_lo:wo_hi + 1, :]
                                        in1 = mx4[:, ::-1, ::-1, :] \
                                            if (nki > 1 or nkj > 1) else mx4[:, 0, 0, :]
                                        nc.vector.tensor_tensor(out=seg, in0=seg,
                                                                in1=in1, op=Alu.add)
                # deferred chain-completion matmuls (chains already finished)
                for n in range(N_LOC):
                    for ho in range(K3):
                        for wo in range(WO3):
                            p = n * (HO3 * WO3) + ho * WO3 + wo
                            off = (ho * WO3 + wo) * C3
                            emit_mm3(p, acc3t[0][n][:, off:off + C3])
                            emit_mm3(p, acc3t[1][n][:, off:off + C3])
                    for ho in range(K3, K3 + KP3):
                        for wo in range(WO3):
                            p = n * (HO3 * WO3) + ho * WO3 + wo
                            off = ((ho - K3) * WO3 + wo) * C3
                            emit_mm3(p, accp3t[0][n][:, off:off + C3])
                            emit_mm3(p, accp3t[1][n][:, off:off + C3])

                nc.vector.scalar_tensor_tensor(
                    out=y3[0:P3, :], in0=pt3[0:P3, :], scalar=xs3col[0:P3, 0:1],
                    in1=wb3[0:P3, :], op0=Alu.add, op1=Alu.add)
                st3_sum = ps3s.tile([1, C3], f32)
                st3_sq = ps3s.tile([1, C3], f32)
                sq3 = sq3p.tile([128, C3], f32)
                nc.scalar.activation(out=sq3[0:P3, :], in_=y3[0:P3, :], func=Act.Square)
                nc.tensor.matmul(st3_sum[0:1, :], lhsT=(ones_k[0:P3, 0:1]),
                                 rhs=(y3[0:P3, :]), start=True, stop=True)
                nc.tensor.matmul(st3_sq[0:1, :], lhsT=(ones_k[0:P3, 0:1]),
                                 rhs=(sq3[0:P3, :]), start=True, stop=True)
                st3_sb = sp.tile([1, 2 * C3], f32)
                nc.vector.tensor_copy(st3_sb[0:1, 0:C3], st3_sum[0:1, :])
                nc.vector.tensor_copy(st3_sb[0:1, C3:2 * C3], st3_sq[0:1, :])
                cc3 = allreduce([st3_sb[0:1, :]], [2 * C3], "bn3")
            st3 = sp.tile([128, 6], f32)
            for cb in range(3):
                nc.gpsimd.dma_start(
                    st3[:, cb:cb + 1],
                    cc3[0:1, cb * 128:(cb + 1) * 128].rearrange("one (p w) -> (one p) w", w=1))
                nc.gpsimd.dma_start(
                    st3[:, 3 + cb:4 + cb],
                    cc3[0:1, C3 + cb * 128:C3 + (cb + 1) * 128].rearrange("one (p w) -> (one p) w", w=1))
            with tc.tile_pool(name="pst3", bufs=3, space="PSUM") as pst3:
                ptr3s = []
                for cb in range(3):
                    ptr3 = pst3.tile([128, P3], f32, tag="pst3", name=f"pst3_{cb}")
                    nc.tensor.transpose(ptr3[:, 0:P3],
                                        y3[0:P3, cb * 128:(cb + 1) * 128],
                                        ident[0:P3, 0:P3])
                    ptr3s.append(ptr3)
                for cb in range(3):
                    sc3, bi3 = bn_affine(st3[:, cb:cb + 1], st3[:, 3 + cb:4 + cb],
                                         gb3[cb], N_TOT * HO3 * WO3, f"bn3_{cb}")
                    nc.scalar.activation(out=act3[:, cb * P3:(cb + 1) * P3],
                                         in_=ptr3s[cb][:, 0:P3], func=Act.Relu,
                                         scale=sc3[:], bias=bi3[:])

            # =================================================== FC + LN + L2
            with tc.tile_pool(name="psfc", bufs=1, space="PSUM") as psfc_p, \
                 tc.tile_pool(name="fcp", bufs=2) as fcp:
                fcacc = sp.tile([128, 12], f32)
                for jj in range(6):
                    for n in range(N_LOC):
                        prod = fcp.tile([128, 3 * 42], f32, tag="prod", name="prod")
                        a3v = act3.rearrange("p (cb q) -> p cb q", cb=3)[:, :, n * 42:(n + 1) * 42]
                        wv = wfc_sb.rearrange("p (j cb q) -> p j cb q", j=6, cb=3)[:, jj]
                        nc.vector.scalar_tensor_tensor(
                            out=prod[:], in0=a3v, scalar=0.0, in1=wv,
                            op0=Alu.add, op1=Alu.mult,
                            accum_out=fcacc[:, jj * 2 + n:jj * 2 + n + 1])
                psfc = psfc_p.tile([1, 12], f32)
                nc.tensor.matmul(psfc[0:1, :], lhsT=ones_k[:, 0:1], rhs=fcacc[:],
                                 start=True, stop=True)
                h12 = sp.tile([1, 12], f32)
                h12v = h12.rearrange("one (j n) -> one j n", n=N_LOC)
                psv = psfc.rearrange("one (j n) -> one j n", n=N_LOC)
                for n in range(N_LOC):
                    nc.vector.tensor_tensor(out=h12v[:, :, n], in0=psv[:, :, n],
                                            in1=bfc_sb[:], op=Alu.add)
                # LN stats
                lnS = sp.tile([1, 1], f32)
                lnQ = sp.tile([1, 1], f32)
                scrl = sp.tile([1, 12], f32)
                nc.vector.tensor_scalar(out=scrl[:], in0=h12[:], scalar1=0.0,
                                        scalar2=None, op0=Alu.add, op1=Alu.add,
                                        accum_out=lnS[:])
                nc.scalar.activation(out=scrl[:], in_=h12[:], func=Act.Square,
                                     accum_out=lnQ[:])
                ccl = allreduce([lnS[:], lnQ[:]], [1, 1], "ln")
                stl = sp.tile([1, 2], f32)
                nc.gpsimd.dma_start(stl[:], ccl[0:1, 0:2])
                mu = sp.tile([1, 1], f32)
                qv = sp.tile([1, 1], f32)
                mu2 = sp.tile([1, 1], f32)
                tvl = sp.tile([1, 1], f32)
                sl_ = sp.tile([1, 1], f32)
                rl0 = sp.tile([1, 1], f32)
                rl0sq = sp.tile([1, 1], f32)
                avl = sp.tile([1, 1], f32)
                bvl = sp.tile([1, 1], f32)
                rl = sp.tile([1, 1], f32)
                inv_tot = 1.0 / (N_TOT * 6)
                nc.vector.tensor_scalar(out=mu[:], in0=stl[:, 0:1], scalar1=inv_tot,
                                        scalar2=None, op0=Alu.mult)
                nc.vector.tensor_scalar(out=qv[:], in0=stl[:, 1:2], scalar1=inv_tot,
                                        scalar2=None, op0=Alu.mult)
                nc.vector.tensor_tensor(out=mu2[:], in0=mu[:], in1=mu[:], op=Alu.mult)
                nc.vector.scalar_tensor_tensor(out=tvl[:], in0=qv[:], scalar=EPS_LN,
                                               in1=mu2[:], op0=Alu.add, op1=Alu.subtract)
                nc.scalar.activation(out=sl_[:], in_=tvl[:], func=Act.Sqrt)
                nc.vector.reciprocal(out=rl0[:], in_=sl_[:])
                nc.vector.tensor_tensor(out=rl0sq[:], in0=rl0[:], in1=rl0[:], op=Alu.mult)
                nc.vector.tensor_tensor(out=avl[:], in0=tvl[:], in1=rl0sq[:], op=Alu.mult)
                nc.vector.tensor_scalar(out=bvl[:], in0=avl[:], scalar1=-0.5,
                                        scalar2=1.5, op0=Alu.mult, op1=Alu.add)
                nc.vector.tensor_tensor(out=rl[:], in0=rl0[:], in1=bvl[:], op=Alu.mult)
                y12 = sp.tile([1, 12], f32)
                nc.vector.tensor_scalar(out=y12[:], in0=h12[:], scalar1=mu[:],
                                        scalar2=rl[:], op0=Alu.subtract, op1=Alu.mult)
                ysq = sp.tile([1, 12], f32)
                nc.scalar.activation(out=ysq[:], in_=y12[:], func=Act.Square)
                out12 = sp.tile([1, 12], f32)
                y12v = y12.rearrange("one (j n) -> one j n", n=N_LOC)
                ysqv = ysq.rearrange("one (j n) -> one j n", n=N_LOC)
                o12v = out12.rearrange("one (j n) -> one j n", n=N_LOC)
                for n in range(N_LOC):
                    nrm = sp.tile([1, 1], f32, name=f"nrm_{n}")
                    srt = sp.tile([1, 1], f32, name=f"srt_{n}")
                    mx_ = sp.tile([1, 1], f32, name=f"mx_{n}")
                    ivn = sp.tile([1, 1], f32, name=f"ivn_{n}")
                    scrn = sp.tile([1, 6], f32, name=f"scrn_{n}")
                    nc.vector.tensor_scalar(out=scrn[:], in0=ysqv[:, :, n], scalar1=0.0,
                                            scalar2=None, op0=Alu.add, op1=Alu.add,
                                            accum_out=nrm[:])
                    nc.scalar.activation(out=srt[:], in_=nrm[:], func=Act.Sqrt)
                    nc.vector.tensor_scalar(out=mx_[:], in0=srt[:], scalar1=EPS_L2,
                                            scalar2=None, op0=Alu.max)
                    nc.vector.reciprocal(out=ivn[:], in_=mx_[:])
                    nc.vector.tensor_scalar(out=o12v[:, :, n], in0=y12v[:, :, n],
                                            scalar1=ivn[:], scalar2=None, op0=Alu.mult)
                outnj = sp.tile([1, 12], f32)
                nc.vector.tensor_copy(
                    outnj.rearrange("one (n j) -> one n j", n=N_LOC),
                    out12.rearrange("one (j n) -> one n j", n=N_LOC))
                nc.gpsimd.dma_start(out_d, outnj[:])

    nc.compile()
    return nc


def _prep_inputs(inputs):
    """Host-side reshapes of the full inputs into per-core in_maps."""
    x = np.asarray(inputs["x"], np.float32)
    w1 = np.asarray(inputs["w1"], np.float32)
    w2 = np.asarray(inputs["w2"], np.float32)
    w3 = np.asarray(inputs["w3"], np.float32)
    Wfc = np.asarray(inputs["Wfc"], np.float32)

    nw1t = (-w1[:, 0, :, 0].T).reshape(1, TAPS1 * C1).copy()          # [1, 6*128]
    w2t = np.ascontiguousarray(w2[:, :, :, 0].transpose(1, 2, 0)).reshape(
        128, TAPS2 * C2).astype(np.float16)
    # w3: (384, 256, 6, 2) -> [ci_in_block, (cib, tap(i,j), c)]
    w3r = w3.transpose(1, 2, 3, 0).reshape(2, 128, TAPS3, C3)          # (cib, ci, tap, c)
    w3t = np.ascontiguousarray(w3r.transpose(1, 0, 2, 3)).reshape(
        128, KB3 * C3).astype(np.float16)
    # Wfc: (6, 16128) with k = c3*42 + ho*2 + wo -> [ci, (j, cb, howo)]
    wf = Wfc.reshape(6, 3, 128, 42)                                    # (j, cb, ci, howo)
    wfcp = np.ascontiguousarray(wf.transpose(2, 0, 1, 3)).reshape(128, 6 * 3 * 42)

    shared = {
        "nw1t": nw1t, "w2t": w2t, "w3t": w3t, "wfcp": wfcp,
        "g1_d": np.asarray(inputs["g1"], np.float32),
        "b1_d": np.asarray(inputs["b1"], np.float32),
        "g2_d": np.asarray(inputs["g2"], np.float32),
        "b2_d": np.asarray(inputs["b2"], np.float32),
        "g3_d": np.asarray(inputs["g3"], np.float32),
        "b3_d": np.asarray(inputs["b3"], np.float32),
        "bfc_d": np.asarray(inputs["bfc"], np.float32),
    }
    in_maps = []
    for i in range(N_CORES):
        m = dict(shared)
        m["x_in"] = np.ascontiguousarray(
            x[i * N_LOC:(i + 1) * N_LOC]).reshape(1, N_LOC * H1 * W1)
        in_maps.append(m)
    return in_maps


def _run(inputs, trace=False):
    if "nc" not in _BUILD_CACHE:
        _BUILD_CACHE["nc"] = build_program()
    nc = _BUILD_CACHE["nc"]
    from concourse import bass_utils
    in_maps = _prep_inputs(inputs)
    res = bass_utils.run_bass_kernel_spmd(
        nc, in_maps, core_ids=list(range(N_CORES)), trace=trace)
    out = np.concatenate(
        [np.asarray(r["out"]).reshape(N_LOC, 6) for r in res.results], axis=0)
    return out, res


def kernel(**inputs):
    return _run(inputs, trace=False)[0]
